# revision 4
# baseline (speedup 1.0000x reference)
"""Trainium2 Bass kernel for nn_DefAddkeysTransformer — v2.

Sharding: one attention head per NeuronCore (8 heads / 8 cores).  Host computes
the exact gather indices (f64 offset matmul + IEEE f32 elementwise, matching
the reference), so the device does: batched dma_gather of deformable keys,
the scramble-aware score matmuls, a max-free softmax, DVE-based weighted value
accumulation, and the head's output contribution in transposed (C, Lq) layout.
Host sums the 8 partial outputs.
"""
import sys

sys.path.insert(0, '/opt/trn_rl_repo')

from contextlib import ExitStack

import numpy as np

import concourse.bass as bass
import concourse.tile as tile
from concourse import bacc
from concourse import mybir
from concourse.bass_utils import run_bass_kernel_spmd
from concourse.masks import make_identity

C = 256
H = 8
L = 4
P = 4
LQ = 1024
LX = 256
LEN_IN = 13294
NT = LQ // 128          # 8 query tiles
NJ = L * P              # 16 (level, point) combos
F32 = mybir.dt.float32
F32R = mybir.dt.float32r
F16 = mybir.dt.float16
I16 = mybir.dt.int16


def build_kernel(nc: bass.Bass, tc: tile.TileContext, ctx: ExitStack, debug=False):
    # ---------------- DRAM I/O ----------------
    d_idx = nc.dram_tensor("idx16", [128, LQ], I16, kind="ExternalInput").ap()
    d_flat = nc.dram_tensor("flatten16", [LEN_IN, C], F16, kind="ExternalInput").ap()
    d_qT = nc.dram_tensor("qT", [C, LQ], F32, kind="ExternalInput").ap()
    d_qT16 = nc.dram_tensor("qT16", [C, LQ], F16, kind="ExternalInput").ap()
    d_wattnT = nc.dram_tensor("wattnT", [5, C, C], F32R, kind="ExternalInput").ap()
    d_addkT = nc.dram_tensor("addkT", [C, LX], F32R, kind="ExternalInput").ap()
    d_wvT16 = nc.dram_tensor("wvT16", [C, C], F16, kind="ExternalInput").ap()
    d_wv2T = nc.dram_tensor("wv2T", [C, C], F32R, kind="ExternalInput").ap()
    d_bvd = nc.dram_tensor("bvd", [1, C], F32R, kind="ExternalInput").ap()
    d_bv2 = nc.dram_tensor("bv2", [C, 1], F32, kind="ExternalInput").ap()
    d_wmix = nc.dram_tensor("wmix_r", [C, 9], F32, kind="ExternalInput").ap()
    d_flag = nc.dram_tensor("flag", [128, 1], F32, kind="ExternalInput").ap()
    d_dmask = nc.dram_tensor("dmask", [128, 512], F32R, kind="ExternalInput").ap()
    d_oh48 = nc.dram_tensor("oh48", [128, 8], F32R, kind="ExternalInput").ap()
    d_out = nc.dram_tensor("outT", [C, LQ], F32, kind="ExternalOutput").ap()
    if debug:
        d_dbg_sall = nc.dram_tensor("dbg_sall", [128, LQ], F32, kind="ExternalOutput").ap()
        d_dbg_wq = nc.dram_tensor("dbg_wq", [128, NT, 16], F32, kind="ExternalOutput").ap()
        d_dbg_v = nc.dram_tensor("dbg_v", [128, NT, C], F32, kind="ExternalOutput").ap()
        d_dbg_z = nc.dram_tensor("dbg_z", [128, NT], F32, kind="ExternalOutput").ap()

    # ---------------- pools ----------------
    cst = ctx.enter_context(tc.tile_pool(name="cst", bufs=1))
    wrk = ctx.enter_context(tc.tile_pool(name="wrk", bufs=3))
    gsp = ctx.enter_context(tc.tile_pool(name="gsp", bufs=2))
    stsb = ctx.enter_context(tc.tile_pool(name="stsb", bufs=2))
    ps_st = ctx.enter_context(tc.tile_pool(name="ps_st", bufs=2, space="PSUM"))
    ps_c = ctx.enter_context(tc.tile_pool(name="ps_c", bufs=2, space="PSUM"))
    ps_tp = ctx.enter_context(tc.tile_pool(name="ps_tp", bufs=2, space="PSUM"))
    ps_o = ctx.enter_context(tc.tile_pool(name="ps_o", bufs=2, space="PSUM"))

    def csttile(shape, dtype=F32, tag=None):
        return cst.tile(shape, dtype, tag=tag, name=tag)

    # ------- phase 0: index load, then the gather spine ---------------------
    IDX = csttile([128, LQ], I16, tag="idx")
    nc.sync.dma_start(IDX[:], d_idx[:])
    G = [csttile([128, NJ * C], F16, tag=f"g{t}") for t in range(NT)]
    for t in range(NT):
        for hf in range(2):
            nc.gpsimd.dma_gather(
                G[t][:, 2048 * hf:2048 * hf + 2048].rearrange(
                    "p (j c) -> p j c", c=C),
                d_flat[:],
                IDX[:, 64 * (2 * t + hf):64 * (2 * t + hf) + 64],
                1024, 1024, C,
            )

    # ------- phase 1: weight loads ------------------------------------------
    QT16 = [csttile([128, LQ], F16, tag=f"qt16_{ch}") for ch in range(2)]
    for ch in range(2):
        nc.sync.dma_start(QT16[ch][:], d_qT16[128 * ch:128 * ch + 128, :])
    SimT = [[csttile([128, C], F32R, tag=f"sim{i}_{ch}") for ch in range(2)]
            for i in range(5)]
    for i in range(5):
        for ch in range(2):
            nc.sync.dma_start(SimT[i][ch][:], d_wattnT[i, 128 * ch:128 * ch + 128, :])
    AddkT = [csttile([128, LX], F32R, tag=f"addkt{ch}") for ch in range(2)]
    WvT16 = [csttile([128, C], F16, tag=f"wvt{ch}") for ch in range(2)]
    Wv2T = [csttile([128, C], F32R, tag=f"wv2t{ch}") for ch in range(2)]
    WM = [csttile([128, 9], tag=f"wm{ch}") for ch in range(2)]
    BV2 = [csttile([128, 1], tag=f"bv2{ch}") for ch in range(2)]
    for ch in range(2):
        sl = slice(128 * ch, 128 * ch + 128)
        nc.sync.dma_start(AddkT[ch][:], d_addkT[sl, :])
        nc.sync.dma_start(WvT16[ch][:], d_wvT16[sl, :])
        nc.sync.dma_start(Wv2T[ch][:], d_wv2T[sl, :])
        nc.sync.dma_start(WM[ch][:], d_wmix[sl, :])
        nc.sync.dma_start(BV2[ch][:], d_bv2[sl, :])
    DMASK = csttile([128, 512], F32R, tag="dmask")
    nc.sync.dma_start(DMASK[:], d_dmask[:])
    OH48 = csttile([128, 8], F32R, tag="oh48")
    nc.sync.dma_start(OH48[:], d_oh48[:])
    FLG = csttile([128, 1], tag="flg")
    nc.sync.dma_start(FLG[:], d_flag[:])
    BVD = csttile([1, C], F32R, tag="bvd")
    nc.sync.dma_start(BVD[:], d_bvd[:])
    QTF = [csttile([128, LQ], tag=f"qtf{ch}") for ch in range(2)]
    for ch in range(2):
        nc.sync.dma_start(QTF[ch][:], d_qT[128 * ch:128 * ch + 128, :])

    IDENT = csttile([128, 128], tag="ident")
    make_identity(nc, IDENT[:])
    NEG16 = csttile([128, 1], tag="neg16")
    nc.vector.memset(NEG16[:], -16.0)
    ONE1F = csttile([1, 128], tag="one1f")
    nc.vector.memset(ONE1F[:], 1.0)
    ONE1 = csttile([1, 128], F32R, tag="one1")
    nc.vector.tensor_copy(ONE1[:], ONE1F[:])
    ONECOLF = csttile([128, 1], tag="onecolf")
    nc.vector.memset(ONECOLF[:], 1.0)
    ONECOL = csttile([128, 1], F32R, tag="onecol")
    nc.vector.tensor_copy(ONECOL[:], ONECOLF[:])

    # head_w softmax over the 9 mixture logits (cols pre-ordered [h, 8, rest])
    HWH = []
    BASE = []
    BV2HW = []
    for ch in range(2):
        mx = wrk.tile([128, 1], F32, tag="mx", name="mx")
        nc.vector.reduce_max(mx[:], WM[ch][:], axis=mybir.AxisListType.X)
        nmx = wrk.tile([128, 1], F32, tag="nmx", name="nmx")
        nc.vector.tensor_scalar_mul(nmx[:], mx[:], -1.0)
        ex = wrk.tile([128, 9], F32, tag="ex", name="ex")
        sm = wrk.tile([128, 1], F32, tag="sm", name="sm")
        nc.scalar.activation(ex[:], WM[ch][:], mybir.ActivationFunctionType.Exp,
                             bias=nmx[:], accum_out=sm[:])
        rs = wrk.tile([128, 1], F32, tag="rs", name="rs")
        nc.vector.reciprocal(rs[:], sm[:])
        hw = csttile([128, 2], tag=f"hw{ch}")
        nc.vector.tensor_scalar_mul(hw[:], ex[:, 0:2], rs[:])
        HWH.append(hw[:, 0:1])
        base = csttile([128, 1], tag=f"base{ch}")
        nc.vector.tensor_tensor(out=base[:], in0=hw[:, 1:2], in1=FLG[:],
                                op=mybir.AluOpType.mult)
        BASE.append(base)
        b2h = csttile([128, 1], tag=f"b2h{ch}")
        nc.vector.tensor_tensor(out=b2h[:], in0=BV2[ch][:], in1=hw[:, 0:1],
                                op=mybir.AluOpType.mult)
        BV2HW.append(b2h)

    # ------- phase 2: add_keys branch (all in transposed orientation) -------
    # KiT[m] = (simil_add @ addk.T) chunk: [128 c'-part (chunk m), Lx]
    KiT = [csttile([128, LX], F16, tag=f"kit{m}") for m in range(2)]
    for m in range(2):
        pps = ps_c.tile([128, LX], F32, tag="pc", name="pc")
        for dch in range(2):
            nc.tensor.matmul(pps[:], lhsT=SimT[4][dch][:, 128 * m:128 * m + 128],
                             rhs=AddkT[dch][:], start=(dch == 0), stop=(dch == 1))
        nc.vector.tensor_copy(KiT[m][:], pps[:])

    # v2 = add_keys @ W_val[2h+1].T   (Lx x C), transposed chunks
    V2 = [csttile([128, C], F32R, tag=f"v2{m}") for m in range(2)]
    for m in range(2):
        pps = ps_c.tile([128, C], F32, tag="pc", name="pc")
        for dch in range(2):
            nc.tensor.matmul(pps[:], lhsT=AddkT[dch][:, 128 * m:128 * m + 128],
                             rhs=Wv2T[dch][:], start=(dch == 0), stop=(dch == 1))
        nc.vector.tensor_copy(V2[m][:], pps[:])

    # WAT[xh] = exp(scoresT - 16): scoresT[x, q] = sum_c Ki[x, c] q[q, c]
    WAT = [csttile([128, LQ], F32R, tag=f"wat{xh}") for xh in range(2)]
    for xh in range(2):
        for qh in range(2):
            qsl = slice(512 * qh, 512 * qh + 512)
            pw = ps_c.tile([128, 512], F32, tag="pc", name="pc")
            for m in range(2):
                nc.tensor.matmul(pw[:], lhsT=KiT[m][:, 128 * xh:128 * xh + 128],
                                 rhs=QT16[m][:, qsl], start=(m == 0), stop=(m == 1))
            nc.scalar.activation(WAT[xh][:, qsl], pw[:],
                                 mybir.ActivationFunctionType.Exp, bias=NEG16[:])

    # SALLB rows 32*lvl+p: level scores; ZADDROW: summed add_keys weights
    SALLB = cst.tile([128, LQ], F32, tag="sallb", name="sallb")
    nc.vector.memset(SALLB[:], 0.0)
    ZADDROW = cst.tile([1, LQ], F32, tag="zaddrow", name="zaddrow")
    for qh in range(2):
        qsl = slice(512 * qh, 512 * qh + 512)
        pz = ps_o.tile([1, 512], F32, tag="po", name="po")
        for xh in range(2):
            nc.tensor.matmul(pz[:], lhsT=ONECOL[:], rhs=WAT[xh][:, qsl],
                             start=(xh == 0), stop=(xh == 1))
        nc.scalar.copy(ZADDROW[:, qsl], pz[:])

    # ------- phase 3: per-level scrambled scores ----------------------------
    for lvl in range(L):
        STB = [[stsb.tile([128, 512], F32R, tag=f"stb{b8}_{dch}",
                          name=f"stb{b8}_{dch}") for dch in range(2)]
               for b8 in range(2)]
        for ql in range(16):
            b, pp = ql % 4, ql // 4
            sps = ps_st.tile([128, 128], F32, tag="pst", name="pst")
            for dch in range(2):
                for c2 in range(2):
                    nc.tensor.matmul(
                        sps[:, 64 * dch:64 * dch + 64],
                        lhsT=G[2 * b + c2][:, (4 * lvl + pp) * 256 + 128 * dch:
                                 (4 * lvl + pp) * 256 + 128 * dch + 128],
                        rhs=QT16[c2][:].rearrange("p (a b) -> p a b", b=16)[:, :, ql],
                        start=(c2 == 0), stop=(c2 == 1))
            qb = ql % 8
            for dch in range(2):
                nc.any.tensor_copy(
                    STB[ql // 8][dch][:, 64 * qb:64 * qb + 64],
                    sps[:, 64 * dch:64 * dch + 64])
        for b8 in range(2):
            scp = ps_o.tile([4, 512], F32, tag="po", name="po")
            for ich in range(2):
                cps = ps_c.tile([128, 512], F32, tag="pc", name="pc")
                for dch in range(2):
                    nc.tensor.matmul(cps[:], lhsT=SimT[lvl][dch][:, 128 * ich:128 * ich + 128],
                                     rhs=STB[b8][dch][:],
                                     start=(dch == 0), stop=(dch == 1))
                mskb = wrk.tile([128, 512], F32R, tag="mskb", name="mskb")
                nc.vector.tensor_tensor(out=mskb[:], in0=cps[:], in1=DMASK[:],
                                        op=mybir.AluOpType.mult)
                nc.tensor.matmul(scp[:], lhsT=OH48[:, 4 * ich:4 * ich + 4],
                                 rhs=mskb[:], start=(ich == 0), stop=(ich == 1))
            sview = SALLB[32 * lvl:32 * lvl + 4, :].rearrange("p (t s) -> p s t", s=16)
            nc.vector.tensor_copy(sview[:, 8 * b8:8 * b8 + 8, :], scp[:])
    if debug:
        nc.sync.dma_start(d_dbg_sall[:], SALLB[:])

    # ------- phase 4: per-tile softmax weights + weighted values ------------
    VT = [cst.tile([128, LQ], F16, tag=f"vt{ch}", name=f"vt{ch}")
          for ch in range(2)]
    ZROW = cst.tile([1, LQ], F32R, tag="zrow", name="zrow")
    S1ZROW = cst.tile([1, LQ], F32R, tag="s1zrow", name="s1zrow")
    for t in range(NT):
        qsl = slice(128 * t, 128 * t + 128)
        tps = ps_tp.tile([128, 128], F32, tag="ptp", name="ptp")
        nc.tensor.transpose(out=tps[:], in_=SALLB[:, qsl], identity=IDENT[:])
        WQ16 = wrk.tile([128, 16], F16, tag="wq16", name="wq16")
        ZL = wrk.tile([128, 1], F32, tag="zl", name="zl")
        sc_view = tps[:].rearrange("p (l r) -> p l r", r=32)[:, :, 0:4]
        nc.scalar.activation(WQ16[:], sc_view,
                             mybir.ActivationFunctionType.Exp,
                             bias=NEG16[:], accum_out=ZL[:])
        tzq = ps_tp.tile([128, 1], F32, tag="ptp", name="ptp")
        nc.tensor.transpose(out=tzq[:], in_=ZADDROW[:, qsl],
                            identity=IDENT[:1, :1])
        zq = wrk.tile([128, 1], F32, tag="zq", name="zq")
        nc.vector.tensor_tensor(out=zq[:], in0=ZL[:], in1=tzq[:],
                                op=mybir.AluOpType.add)
        RS2 = wrk.tile([128, 2], F32, tag="rs2", name="rs2")
        nc.vector.reciprocal(RS2[:, 0:1], zq[:])
        nc.vector.tensor_tensor(out=RS2[:, 1:2], in0=ZL[:], in1=RS2[:, 0:1],
                                op=mybir.AluOpType.mult)
        if debug:
            nc.sync.dma_start(d_dbg_wq[:, t, :], WQ16[:])
            nc.sync.dma_start(d_dbg_z[:, t:t + 1], zq[:])
        GS = gsp.tile([128, NJ * C], F16, tag="gs", name="gs")
        nc.vector.tensor_tensor(
            out=GS[:].rearrange("p (j c) -> p j c", c=C),
            in0=G[t][:].rearrange("p (j c) -> p j c", c=C),
            in1=WQ16[:].unsqueeze(2).broadcast_to((128, NJ, C)),
            op=mybir.AluOpType.mult)
        RED = wrk.tile([128, C], F32, tag="red", name="red")
        nc.vector.tensor_reduce(
            out=RED[:], in_=GS[:].rearrange("p (j c) -> p c j", c=C),
            axis=mybir.AxisListType.X, op=mybir.AluOpType.add)
        if debug:
            nc.sync.dma_start(d_dbg_v[:, t, :], RED[:])
        VZ = wrk.tile([128, C], F32, tag="vz", name="vz")
        nc.scalar.activation(VZ[:], RED[:], mybir.ActivationFunctionType.Copy,
                             scale=RS2[:, 0:1])
        for ch in range(2):
            tp2 = ps_tp.tile([128, 128], F32, tag="ptp", name="ptp")
            nc.tensor.transpose(out=tp2[:], in_=VZ[:, 128 * ch:128 * ch + 128],
                                identity=IDENT[:])
            nc.vector.tensor_copy(VT[ch][:, qsl], tp2[:])
        tz = ps_tp.tile([1, 128], F32, tag="ptp", name="ptp")
        nc.tensor.transpose(out=tz[:], in_=RS2[:, 0:1], identity=IDENT[:])
        nc.vector.tensor_copy(ZROW[:, qsl], tz[:])
        tz2 = ps_tp.tile([1, 128], F32, tag="ptp", name="ptp")
        nc.tensor.transpose(out=tz2[:], in_=RS2[:, 1:2], identity=IDENT[:])
        nc.vector.tensor_copy(S1ZROW[:, qsl], tz2[:])

    # ------- phase 5: output matmuls ----------------------------------------
    RZB = cst.tile([128, LQ], F32, tag="rzb", name="rzb")
    for n in range(2):
        nsl = slice(512 * n, 512 * n + 512)
        rzp = ps_c.tile([128, 512], F32, tag="pc", name="pc")
        nc.tensor.matmul(rzp[:], lhsT=ONE1[:], rhs=ZROW[:, nsl],
                         start=True, stop=True)
        nc.vector.tensor_copy(RZB[:, nsl], rzp[:])
    RES = [cst.tile([128, LQ], F32, tag=f"res{m}", name=f"res{m}") for m in range(2)]
    for m in range(2):
        msl = slice(128 * m, 128 * m + 128)
        for n in range(2):
            nsl = slice(512 * n, 512 * n + 512)
            ops1 = ps_o.tile([128, 512], F32, tag="po", name="po")
            nc.tensor.matmul(ops1[:], lhsT=WvT16[0][:, msl], rhs=VT[0][:, nsl],
                             start=True, stop=False)
            nc.tensor.matmul(ops1[:], lhsT=WvT16[1][:, msl], rhs=VT[1][:, nsl],
                             start=False, stop=False)
            nc.tensor.matmul(ops1[:], lhsT=BVD[:, msl], rhs=S1ZROW[:, nsl],
                             start=False, stop=True)
            ops2 = ps_st.tile([128, 512], F32, tag="pst", name="pst")
            nc.tensor.matmul(ops2[:], lhsT=V2[0][:, msl], rhs=WAT[0][:, nsl],
                             start=True, stop=False)
            nc.tensor.matmul(ops2[:], lhsT=V2[1][:, msl], rhs=WAT[1][:, nsl],
                             start=False, stop=True)
            sc1 = wrk.tile([128, 512], F32, tag="sc1", name="sc1")
            nc.vector.tensor_tensor(out=sc1[:], in0=ops2[:], in1=RZB[:, nsl],
                                    op=mybir.AluOpType.mult)
            nc.vector.tensor_tensor(out=sc1[:], in0=sc1[:], in1=ops1[:],
                                    op=mybir.AluOpType.add)
            sc2 = wrk.tile([128, 512], F32, tag="sc2", name="sc2")
            nc.scalar.activation(sc2[:], sc1[:],
                                 mybir.ActivationFunctionType.Copy, scale=HWH[m])
            bt = wrk.tile([128, 512], F32, tag="bt", name="bt")
            nc.scalar.activation(bt[:], QTF[m][:, nsl],
                                 mybir.ActivationFunctionType.Copy, scale=BASE[m][:])
            nc.vector.tensor_tensor(out=sc2[:], in0=sc2[:], in1=bt[:],
                                    op=mybir.AluOpType.add)
            nc.vector.tensor_scalar_add(RES[m][:, nsl], sc2[:], BV2HW[m][:])
        nc.sync.dma_start(d_out[msl, :], RES[m][:])


def _host_indices(inputs):
    """Exact replica of the reference index computation (f64 matmul, IEEE f32
    elementwise).  Returns flat indices [Lq, H, L, P] int32."""
    q = np.asarray(inputs["query"], np.float32)[0]            # (1024, 256)
    rp = np.asarray(inputs["reference_points"], np.float32)[0]  # (1024, 4, 2)
    iss = np.asarray(inputs["input_spatial_shapes"], np.int32)
    lvst = np.asarray(inputs["input_level_start_index"], np.int32)
    W_off = np.asarray(inputs["W_off"], np.float32)
    b_off = np.asarray(inputs["b_off"], np.float32)

    off = (q.astype(np.float64) @ W_off.T.astype(np.float64)).astype(np.float32)
    off = off + b_off[None, :]
    off = off.reshape(LQ, H, L, P, 2)
    iss_f = iss.astype(np.float32)
    wh = iss_f[:, ::-1]                                       # (L, 2) = (W, H)
    loc = rp[:, None, :, None, :] + off / wh[None, None, :, None, :]
    loc = np.clip(loc, np.float32(0.0), np.float32(0.999))
    idx = (loc * iss_f[None, None, :, None, :]).astype(np.int32)
    h_l = iss[:, 0]
    flat = (idx[..., 0] + idx[..., 1] * h_l[None, None, :, None]
            + lvst[None, None, :, None])                      # (Lq, H, L, P)
    return flat


def _host_prepare(inputs):
    """Build per-core input maps from the full problem inputs."""
    q = np.asarray(inputs["query"], np.float32)[0]            # (1024, 256)
    flat_in = np.ascontiguousarray(np.asarray(inputs["input_flatten"], np.float32)[0])
    addk = np.asarray(inputs["add_keys"], np.float32)[0]
    W_attn = np.asarray(inputs["W_attn"], np.float32)
    W_val = np.asarray(inputs["W_val"], np.float32)
    b_val = np.asarray(inputs["b_val"], np.float32)
    W_mix = np.asarray(inputs["W_mix"], np.float32)

    flat = _host_indices(inputs)                              # (Lq, H, L, P)

    ones128 = np.ones((128, 1), np.float32)
    common = {
        "qT": np.ascontiguousarray(q.T),
        "qT16": np.ascontiguousarray(q.T).astype(np.float16),
        "flatten16": flat_in.astype(np.float16),
        "addkT": np.ascontiguousarray(addk.T),
    }
    # diag extraction mask: rows r=(ql%2)*64+t, cols p*64+t' -> 1 iff t'==r%64
    dm = np.zeros((128, 512), np.float32)
    for rr in range(128):
        dm[rr, rr % 64::64] = 1.0
    common["dmask"] = dm
    oh = np.zeros((128, 8), np.float32)
    for rr in range(128):
        oh[rr, rr // 64] = 1.0          # ich 0: i//64 = p
        oh[rr, 4 + 2 + rr // 64] = 1.0  # ich 1: p = 2 + i'//64
    common["oh48"] = oh

    in_maps = []
    for h in range(H):
        fl = flat[:, h].reshape(LQ, NJ).astype(np.int16)      # (1024, 16) j=4*lvl+p
        idx16 = np.zeros((16, LQ), np.int16)
        for t in range(NT):
            for hf in range(2):
                blk = fl[128 * t:128 * t + 128, 8 * hf:8 * hf + 8]  # (128 q, 8 j)
                k = 2 * t + hf
                idx16[:, 64 * k:64 * k + 64] = \
                    blk.T.ravel().reshape(64, 16).T           # wrapped layout
        idx16 = np.tile(idx16, (8, 1))  # Q7 tx/rx cores read their own 16-group
        order = [h, 8] + [k for k in range(9) if k not in (h, 8)]
        m = dict(common)
        m["idx16"] = idx16
        m["wattnT"] = np.ascontiguousarray(
            np.transpose(W_attn[4 * h:4 * h + 5], (0, 2, 1)))
        m["wvT16"] = np.ascontiguousarray(W_val[2 * h].T).astype(np.float16)
        m["wv2T"] = np.ascontiguousarray(W_val[2 * h + 1].T)
        m["bvd"] = (b_val[2 * h] - b_val[2 * h + 1]).reshape(1, C).astype(np.float32)
        m["bv2"] = b_val[2 * h + 1].reshape(C, 1).astype(np.float32)
        m["wmix_r"] = np.ascontiguousarray(W_mix[:, order])
        m["flag"] = ones128 * (1.0 if h == 0 else 0.0)
        in_maps.append(m)
    return in_maps


_CACHE = {}


def _get_nc():
    if "nc" not in _CACHE:
        nc = bacc.Bacc("TRN2", target_bir_lowering=False, debug=False)
        with tile.TileContext(nc) as tc:
            with ExitStack() as ctx:
                build_kernel(nc, tc, ctx)
        nc.compile()
        _CACHE["nc"] = nc
    return _CACHE["nc"]


def kernel(**inputs):
    nc = _get_nc()
    in_maps = _host_prepare(inputs)
    res = run_bass_kernel_spmd(nc, in_maps, core_ids=list(range(8)))
    total = np.zeros((C, LQ), np.float32)
    for h in range(H):
        total = total + res.results[h]["outT"]
    return np.ascontiguousarray(total.T)[None].astype(np.float32)


# revision 5
# speedup vs baseline: 1.4676x; 1.4676x over previous
"""Trainium2 Bass kernel for nn_DefAddkeysTransformer — v2.

Sharding: one attention head per NeuronCore (8 heads / 8 cores).  Host computes
the exact gather indices (f64 offset matmul + IEEE f32 elementwise, matching
the reference), so the device does: batched dma_gather of deformable keys,
the scramble-aware score matmuls, a max-free softmax, DVE-based weighted value
accumulation, and the head's output contribution in transposed (C, Lq) layout.
Host sums the 8 partial outputs.
"""
import sys

sys.path.insert(0, '/opt/trn_rl_repo')

from contextlib import ExitStack

import numpy as np

import concourse.bass as bass
import concourse.tile as tile
from concourse import bacc
from concourse import mybir
from concourse.bass_utils import run_bass_kernel_spmd
C = 256
H = 8
L = 4
P = 4
LQ = 1024
LX = 256
LEN_IN = 13294
NT = LQ // 128          # 8 query tiles
NJ = L * P              # 16 (level, point) combos
F32 = mybir.dt.float32
F32R = mybir.dt.float32r
F16 = mybir.dt.float16
I16 = mybir.dt.int16


def build_kernel(nc: bass.Bass, tc: tile.TileContext, ctx: ExitStack, debug=False):
    # ---------------- DRAM I/O ----------------
    d_idx = nc.dram_tensor("idx16", [128, 768], I16, kind="ExternalInput").ap()
    d_ident = nc.dram_tensor("ident", [128, 128], F32, kind="ExternalInput").ap()
    d_oh3 = nc.dram_tensor("oh3", [128, 8192], F16, kind="ExternalInput").ap()
    d_f3 = nc.dram_tensor("f3pad", [256, C], F16, kind="ExternalInput").ap()
    d_flat = nc.dram_tensor("flatten16", [LEN_IN, C], F16, kind="ExternalInput").ap()
    d_qT16 = nc.dram_tensor("qT16", [C, LQ], F16, kind="ExternalInput").ap()
    d_wattnT = nc.dram_tensor("wattnT", [5, C, C], F32R, kind="ExternalInput").ap()
    d_addkT = nc.dram_tensor("addkT", [C, LX], F32R, kind="ExternalInput").ap()
    d_wvT16 = nc.dram_tensor("wvT16", [C, C], F16, kind="ExternalInput").ap()
    d_wv2T = nc.dram_tensor("wv2T", [C, C], F32R, kind="ExternalInput").ap()
    d_bvd = nc.dram_tensor("bvd", [1, C], F32R, kind="ExternalInput").ap()
    d_bv2 = nc.dram_tensor("bv2", [C, 1], F32, kind="ExternalInput").ap()
    d_wmix = nc.dram_tensor("wmix_r", [C, 9], F32, kind="ExternalInput").ap()
    d_flag = nc.dram_tensor("flag", [128, 1], F32, kind="ExternalInput").ap()
    d_dmask = nc.dram_tensor("dmask", [128, 512], F32R, kind="ExternalInput").ap()
    d_oh48 = nc.dram_tensor("oh48", [128, 8], F32R, kind="ExternalInput").ap()
    d_out = nc.dram_tensor("outT", [C, LQ], F32, kind="ExternalOutput").ap()
    if debug:
        d_dbg_sall = nc.dram_tensor("dbg_sall", [128, LQ], F32, kind="ExternalOutput").ap()
        d_dbg_wq = nc.dram_tensor("dbg_wq", [128, NT, 16], F32, kind="ExternalOutput").ap()
        d_dbg_v = nc.dram_tensor("dbg_v", [128, NT, C], F32, kind="ExternalOutput").ap()
        d_dbg_z = nc.dram_tensor("dbg_z", [128, NT], F32, kind="ExternalOutput").ap()

    # ---------------- pools ----------------
    cst = ctx.enter_context(tc.tile_pool(name="cst", bufs=1))
    wrk = ctx.enter_context(tc.tile_pool(name="wrk", bufs=2))
    gsp = ctx.enter_context(tc.tile_pool(name="gsp", bufs=2))
    stsb = ctx.enter_context(tc.tile_pool(name="stsb", bufs=2))
    ps_st = ctx.enter_context(tc.tile_pool(name="ps_st", bufs=2, space="PSUM"))
    ps_c = ctx.enter_context(tc.tile_pool(name="ps_c", bufs=2, space="PSUM"))
    ps_tp = ctx.enter_context(tc.tile_pool(name="ps_tp", bufs=2, space="PSUM"))
    ps_o = ctx.enter_context(tc.tile_pool(name="ps_o", bufs=2, space="PSUM"))

    def csttile(shape, dtype=F32, tag=None):
        return cst.tile(shape, dtype, tag=tag, name=tag)

    # ------- phase 0: index load, then the gather spine ---------------------
    IDX = csttile([128, 768], I16, tag="idx")
    nc.sync.dma_start(IDX[:], d_idx[:])
    G = [csttile([128, NJ * C], F16, tag=f"g{t}") for t in range(NT)]
    for t in range(NT):
        nc.gpsimd.dma_gather(
            G[t][:, 0:2048].rearrange("p (j c) -> p j c", c=C),
            d_flat[:], IDX[:, 64 * t:64 * t + 64], 1024, 1024, C)
    for t in range(NT):
        nc.gpsimd.dma_gather(
            G[t][:, 2048:3072].rearrange("p (j c) -> p j c", c=C),
            d_flat[:], IDX[:, 512 + 32 * t:512 + 32 * t + 32], 512, 512, C)

    # ------- phase 1: weight loads ------------------------------------------
    QT16 = [csttile([128, LQ], F16, tag=f"qt16_{ch}") for ch in range(2)]
    for ch in range(2):
        nc.sync.dma_start(QT16[ch][:], d_qT16[128 * ch:128 * ch + 128, :])
    SimT = [[csttile([128, C], F32R, tag=f"sim{i}_{ch}") for ch in range(2)]
            for i in range(5)]
    for i in range(5):
        for ch in range(2):
            nc.sync.dma_start(SimT[i][ch][:], d_wattnT[i, 128 * ch:128 * ch + 128, :])
    AddkT = [csttile([128, LX], F32R, tag=f"addkt{ch}") for ch in range(2)]
    WvT16 = [csttile([128, C], F16, tag=f"wvt{ch}") for ch in range(2)]
    Wv2T = [csttile([128, C], F32R, tag=f"wv2t{ch}") for ch in range(2)]
    WM = [csttile([128, 9], tag=f"wm{ch}") for ch in range(2)]
    BV2 = [csttile([128, 1], tag=f"bv2{ch}") for ch in range(2)]
    for ch in range(2):
        sl = slice(128 * ch, 128 * ch + 128)
        nc.sync.dma_start(AddkT[ch][:], d_addkT[sl, :])
        nc.sync.dma_start(WvT16[ch][:], d_wvT16[sl, :])
        nc.sync.dma_start(Wv2T[ch][:], d_wv2T[sl, :])
        nc.sync.dma_start(WM[ch][:], d_wmix[sl, :])
        nc.sync.dma_start(BV2[ch][:], d_bv2[sl, :])
    DMASK = csttile([128, 512], F32R, tag="dmask")
    nc.sync.dma_start(DMASK[:], d_dmask[:])
    OH48 = csttile([128, 8], F32R, tag="oh48")
    nc.sync.dma_start(OH48[:], d_oh48[:])
    FLG = csttile([128, 1], tag="flg")
    nc.sync.dma_start(FLG[:], d_flag[:])
    BVD = csttile([1, C], F32R, tag="bvd")
    nc.sync.dma_start(BVD[:], d_bvd[:])

    IDENT = csttile([128, 128], tag="ident")
    nc.sync.dma_start(IDENT[:], d_ident[:])
    OH3 = csttile([128, 8192], F16, tag="oh3")
    nc.sync.dma_start(OH3[:], d_oh3[:])
    F3T = [csttile([128, C], F16, tag=f"f3{ch}") for ch in range(2)]
    for ch in range(2):
        nc.sync.dma_start(F3T[ch][:], d_f3[128 * ch:128 * ch + 128, :])
    ohv = OH3[:].rearrange("p (tj ch q) -> p tj ch q", ch=2, q=128)
    for t in range(NT):
        for jj in range(4):
            pg = ps_c.tile([128, C], F32, tag="pc", name="pc")
            for ch in range(2):
                nc.tensor.matmul(pg[:], lhsT=ohv[:, 4 * t + jj, ch, :],
                                 rhs=F3T[ch][:], start=(ch == 0), stop=(ch == 1))
            nc.vector.tensor_copy(
                G[t][:, (12 + jj) * 256:(12 + jj) * 256 + 256], pg[:])
    NEG16 = csttile([128, 1], tag="neg16")
    nc.vector.memset(NEG16[:], -16.0)
    NEGB = csttile([128, 1], tag="negb")
    nc.vector.memset(NEGB[:], -16.0 - 4.852030263919617)  # -16 - ln(128)
    ONE1F = csttile([1, 128], tag="one1f")
    nc.vector.memset(ONE1F[:], 1.0)
    ONE1 = csttile([1, 128], F32R, tag="one1")
    nc.vector.tensor_copy(ONE1[:], ONE1F[:])
    ONECOLF = csttile([128, 1], tag="onecolf")
    nc.vector.memset(ONECOLF[:], 1.0)
    ONECOL = csttile([128, 1], F32R, tag="onecol")
    nc.vector.tensor_copy(ONECOL[:], ONECOLF[:])

    # head_w softmax over the 9 mixture logits (cols pre-ordered [h, 8, rest])
    HWH = []
    BASE = []
    BV2HW = []
    for ch in range(2):
        mx = wrk.tile([128, 1], F32, tag="mx", name="mx")
        nc.vector.reduce_max(mx[:], WM[ch][:], axis=mybir.AxisListType.X)
        nmx = wrk.tile([128, 1], F32, tag="nmx", name="nmx")
        nc.vector.tensor_scalar_mul(nmx[:], mx[:], -1.0)
        ex = wrk.tile([128, 9], F32, tag="ex", name="ex")
        sm = wrk.tile([128, 1], F32, tag="sm", name="sm")
        nc.scalar.activation(ex[:], WM[ch][:], mybir.ActivationFunctionType.Exp,
                             bias=nmx[:], accum_out=sm[:])
        rs = wrk.tile([128, 1], F32, tag="rs", name="rs")
        nc.vector.reciprocal(rs[:], sm[:])
        hw = csttile([128, 2], tag=f"hw{ch}")
        nc.vector.tensor_scalar_mul(hw[:], ex[:, 0:2], rs[:])
        HWH.append(hw[:, 0:1])
        base = csttile([128, 1], tag=f"base{ch}")
        nc.vector.tensor_tensor(out=base[:], in0=hw[:, 1:2], in1=FLG[:],
                                op=mybir.AluOpType.mult)
        BASE.append(base)
        b2h = csttile([128, 1], tag=f"b2h{ch}")
        nc.vector.tensor_tensor(out=b2h[:], in0=BV2[ch][:], in1=hw[:, 0:1],
                                op=mybir.AluOpType.mult)
        BV2HW.append(b2h)

    # ------- phase 2: add_keys branch (all in transposed orientation) -------
    # KiT[m] = (simil_add @ addk.T) chunk: [128 c'-part (chunk m), Lx]
    KiT = [csttile([128, LX], F16, tag=f"kit{m}") for m in range(2)]
    for m in range(2):
        pps = ps_c.tile([128, LX], F32, tag="pc", name="pc")
        for dch in range(2):
            nc.tensor.matmul(pps[:], lhsT=SimT[4][dch][:, 128 * m:128 * m + 128],
                             rhs=AddkT[dch][:], start=(dch == 0), stop=(dch == 1))
        nc.vector.tensor_copy(KiT[m][:], pps[:])

    # v2 = add_keys @ W_val[2h+1].T   (Lx x C), transposed chunks
    V2 = [csttile([128, C], F32R, tag=f"v2{m}") for m in range(2)]
    for m in range(2):
        pps = ps_c.tile([128, C], F32, tag="pc", name="pc")
        for dch in range(2):
            nc.tensor.matmul(pps[:], lhsT=AddkT[dch][:, 128 * m:128 * m + 128],
                             rhs=Wv2T[dch][:], start=(dch == 0), stop=(dch == 1))
        nc.vector.tensor_copy(V2[m][:], pps[:])

    # WAT[xh] = exp(scoresT - 16): scoresT[x, q] = sum_c Ki[x, c] q[q, c]
    WAT = [csttile([128, LQ], F32R, tag=f"wat{xh}") for xh in range(2)]
    for xh in range(2):
        for qh in range(2):
            qsl = slice(512 * qh, 512 * qh + 512)
            pw = ps_c.tile([128, 512], F32, tag="pc", name="pc")
            for m in range(2):
                nc.tensor.matmul(pw[:], lhsT=KiT[m][:, 128 * xh:128 * xh + 128],
                                 rhs=QT16[m][:, qsl], start=(m == 0), stop=(m == 1))
            nc.scalar.activation(WAT[xh][:, qsl], pw[:],
                                 mybir.ActivationFunctionType.Exp, bias=NEG16[:])

    # SALLB rows 32*lvl+p: level scores; ZADDROW: summed add_keys weights
    SALLB = cst.tile([128, LQ], F32, tag="sallb", name="sallb")
    nc.vector.memset(SALLB[:], 0.0)
    ZADDROW = cst.tile([1, LQ], F32, tag="zaddrow", name="zaddrow")
    for qh in range(2):
        qsl = slice(512 * qh, 512 * qh + 512)
        pz = ps_o.tile([1, 512], F32, tag="po", name="po")
        for xh in range(2):
            nc.tensor.matmul(pz[:], lhsT=ONECOL[:], rhs=WAT[xh][:, qsl],
                             start=(xh == 0), stop=(xh == 1))
        nc.scalar.copy(ZADDROW[:, qsl], pz[:])

    # ------- phase 3: per-level scrambled scores ----------------------------
    for lvl in (3, 0, 1, 2):
        STB = [[stsb.tile([128, 512], F32R, tag=f"stb{b8}_{dch}",
                          name=f"stb{b8}_{dch}") for dch in range(2)]
               for b8 in range(2)]
        for b in range(4):
          for pp in range(4):
            ql = 4 * pp + b
            sps = ps_st.tile([128, 128], F32, tag="pst", name="pst")
            for dch in range(2):
                for c2 in range(2):
                    nc.tensor.matmul(
                        sps[:, 64 * dch:64 * dch + 64],
                        lhsT=G[2 * b + c2][:, (4 * lvl + pp) * 256 + 128 * dch:
                                 (4 * lvl + pp) * 256 + 128 * dch + 128],
                        rhs=QT16[c2][:].rearrange("p (a b) -> p a b", b=16)[:, :, ql],
                        start=(c2 == 0), stop=(c2 == 1))
            qb = ql % 8
            for dch in range(2):
                nc.scalar.copy(
                    STB[ql // 8][dch][:, 64 * qb:64 * qb + 64],
                    sps[:, 64 * dch:64 * dch + 64])
        for b8 in range(2):
            scp = ps_o.tile([4, 512], F32, tag="po", name="po")
            for ich in range(2):
                cps = ps_c.tile([128, 512], F32, tag="pc", name="pc")
                for dch in range(2):
                    nc.tensor.matmul(cps[:], lhsT=SimT[lvl][dch][:, 128 * ich:128 * ich + 128],
                                     rhs=STB[b8][dch][:],
                                     start=(dch == 0), stop=(dch == 1))
                mskb = wrk.tile([128, 512], F32R, tag="mskb", name="mskb")
                nc.vector.tensor_tensor(out=mskb[:], in0=cps[:], in1=DMASK[:],
                                        op=mybir.AluOpType.mult)
                nc.tensor.matmul(scp[:], lhsT=OH48[:, 4 * ich:4 * ich + 4],
                                 rhs=mskb[:], start=(ich == 0), stop=(ich == 1))
            sview = SALLB[32 * lvl:32 * lvl + 4, :].rearrange("p (t s) -> p s t", s=16)
            nc.vector.tensor_copy(sview[:, 8 * b8:8 * b8 + 8, :], scp[:])
    if debug:
        nc.sync.dma_start(d_dbg_sall[:], SALLB[:])

    # ------- phase 4: per-tile softmax weights + weighted values ------------
    VT = [cst.tile([128, LQ], F16, tag=f"vt{ch}", name=f"vt{ch}")
          for ch in range(2)]
    ZROW = cst.tile([1, LQ], F32R, tag="zrow", name="zrow")
    S1ZROW = cst.tile([1, LQ], F32R, tag="s1zrow", name="s1zrow")
    for t in range(NT):
        qsl = slice(128 * t, 128 * t + 128)
        tps = ps_tp.tile([128, 128], F32, tag="ptp", name="ptp")
        nc.tensor.transpose(out=tps[:], in_=SALLB[:, qsl], identity=IDENT[:])
        WQ16 = wrk.tile([128, 16], F32, tag="wq16", name="wq16")
        ZL = wrk.tile([128, 1], F32, tag="zl", name="zl")
        sc_view = tps[:].rearrange("p (l r) -> p l r", r=32)[:, :, 0:4]
        nc.scalar.activation(WQ16[:], sc_view,
                             mybir.ActivationFunctionType.Exp,
                             bias=NEGB[:], accum_out=ZL[:])
        zl128 = wrk.tile([128, 1], F32, tag="zl128", name="zl128")
        nc.vector.tensor_scalar_mul(zl128[:], ZL[:], 128.0)
        tzq = ps_tp.tile([128, 1], F32, tag="ptp", name="ptp")
        nc.tensor.transpose(out=tzq[:], in_=ZADDROW[:, qsl],
                            identity=IDENT[:1, :1])
        zq = wrk.tile([128, 1], F32, tag="zq", name="zq")
        nc.vector.tensor_tensor(out=zq[:], in0=zl128[:], in1=tzq[:],
                                op=mybir.AluOpType.add)
        RS2 = wrk.tile([128, 2], F32, tag="rs2", name="rs2")
        nc.vector.reciprocal(RS2[:, 0:1], zq[:])
        nc.vector.tensor_tensor(out=RS2[:, 1:2], in0=zl128[:], in1=RS2[:, 0:1],
                                op=mybir.AluOpType.mult)
        rs128 = wrk.tile([128, 1], F32, tag="rs128", name="rs128")
        nc.vector.tensor_scalar_mul(rs128[:], RS2[:, 0:1], 128.0)
        if debug:
            nc.sync.dma_start(d_dbg_wq[:, t, :], WQ16[:])
            nc.sync.dma_start(d_dbg_z[:, t:t + 1], zq[:])
        GS = gsp.tile([128, NJ * C], F16, tag="gs", name="gs")
        nc.vector.tensor_tensor(
            out=GS[:].rearrange("p (j c) -> p j c", c=C),
            in0=G[t][:].rearrange("p (j c) -> p j c", c=C),
            in1=WQ16[:].unsqueeze(2).broadcast_to((128, NJ, C)),
            op=mybir.AluOpType.mult)
        nc.vector.tensor_tensor(out=GS[:, 0:2048], in0=GS[:, 0:2048],
                                in1=GS[:, 2048:4096], op=mybir.AluOpType.add)
        nc.vector.tensor_tensor(out=GS[:, 0:1024], in0=GS[:, 0:1024],
                                in1=GS[:, 1024:2048], op=mybir.AluOpType.add)
        nc.vector.tensor_tensor(out=GS[:, 0:512], in0=GS[:, 0:512],
                                in1=GS[:, 512:1024], op=mybir.AluOpType.add)
        RED = wrk.tile([128, C], F32, tag="red", name="red")
        nc.vector.tensor_tensor(out=RED[:], in0=GS[:, 0:256], in1=GS[:, 256:512],
                                op=mybir.AluOpType.add)
        if debug:
            nc.sync.dma_start(d_dbg_v[:, t, :], RED[:])
        VZ = wrk.tile([128, C], F32, tag="vz", name="vz")
        nc.scalar.activation(VZ[:], RED[:], mybir.ActivationFunctionType.Copy,
                             scale=rs128[:])
        for ch in range(2):
            tp2 = ps_tp.tile([128, 128], F32, tag="ptp", name="ptp")
            nc.tensor.transpose(out=tp2[:], in_=VZ[:, 128 * ch:128 * ch + 128],
                                identity=IDENT[:])
            nc.vector.tensor_copy(VT[ch][:, qsl], tp2[:])
        tz = ps_tp.tile([1, 128], F32, tag="ptp", name="ptp")
        nc.tensor.transpose(out=tz[:], in_=RS2[:, 0:1], identity=IDENT[:])
        nc.vector.tensor_copy(ZROW[:, qsl], tz[:])
        tz2 = ps_tp.tile([1, 128], F32, tag="ptp", name="ptp")
        nc.tensor.transpose(out=tz2[:], in_=RS2[:, 1:2], identity=IDENT[:])
        nc.vector.tensor_copy(S1ZROW[:, qsl], tz2[:])

    # ------- phase 5: output matmuls ----------------------------------------
    RZB = cst.tile([128, LQ], F32, tag="rzb", name="rzb")
    for n in range(2):
        nsl = slice(512 * n, 512 * n + 512)
        rzp = ps_c.tile([128, 512], F32, tag="pc", name="pc")
        nc.tensor.matmul(rzp[:], lhsT=ONE1[:], rhs=ZROW[:, nsl],
                         start=True, stop=True)
        nc.vector.tensor_copy(RZB[:, nsl], rzp[:])
    RES = [cst.tile([128, LQ], F32, tag=f"res{m}", name=f"res{m}") for m in range(2)]
    for m in range(2):
        msl = slice(128 * m, 128 * m + 128)
        for n in range(2):
            nsl = slice(512 * n, 512 * n + 512)
            ops1 = ps_o.tile([128, 512], F32, tag="po", name="po")
            nc.tensor.matmul(ops1[:], lhsT=WvT16[0][:, msl], rhs=VT[0][:, nsl],
                             start=True, stop=False)
            nc.tensor.matmul(ops1[:], lhsT=WvT16[1][:, msl], rhs=VT[1][:, nsl],
                             start=False, stop=False)
            nc.tensor.matmul(ops1[:], lhsT=BVD[:, msl], rhs=S1ZROW[:, nsl],
                             start=False, stop=True)
            ops2 = ps_st.tile([128, 512], F32, tag="pst", name="pst")
            nc.tensor.matmul(ops2[:], lhsT=V2[0][:, msl], rhs=WAT[0][:, nsl],
                             start=True, stop=False)
            nc.tensor.matmul(ops2[:], lhsT=V2[1][:, msl], rhs=WAT[1][:, nsl],
                             start=False, stop=True)
            sc1 = wrk.tile([128, 512], F32, tag="sc1", name="sc1")
            nc.vector.tensor_tensor(out=sc1[:], in0=ops2[:], in1=RZB[:, nsl],
                                    op=mybir.AluOpType.mult)
            nc.vector.tensor_tensor(out=sc1[:], in0=sc1[:], in1=ops1[:],
                                    op=mybir.AluOpType.add)
            sc2 = wrk.tile([128, 512], F32, tag="sc2", name="sc2")
            nc.scalar.activation(sc2[:], sc1[:],
                                 mybir.ActivationFunctionType.Copy, scale=HWH[m])
            bt = wrk.tile([128, 512], F32, tag="bt", name="bt")
            nc.scalar.activation(bt[:], QT16[m][:, nsl],
                                 mybir.ActivationFunctionType.Copy, scale=BASE[m][:])
            nc.vector.tensor_tensor(out=sc2[:], in0=sc2[:], in1=bt[:],
                                    op=mybir.AluOpType.add)
            nc.vector.tensor_scalar_add(RES[m][:, nsl], sc2[:], BV2HW[m][:])
        nc.sync.dma_start(d_out[msl, :], RES[m][:])


def _host_indices(inputs):
    """Exact replica of the reference index computation (f64 matmul, IEEE f32
    elementwise).  Returns flat indices [Lq, H, L, P] int32."""
    q = np.asarray(inputs["query"], np.float32)[0]            # (1024, 256)
    rp = np.asarray(inputs["reference_points"], np.float32)[0]  # (1024, 4, 2)
    iss = np.asarray(inputs["input_spatial_shapes"], np.int32)
    lvst = np.asarray(inputs["input_level_start_index"], np.int32)
    W_off = np.asarray(inputs["W_off"], np.float32)
    b_off = np.asarray(inputs["b_off"], np.float32)

    off = (q.astype(np.float64) @ W_off.T.astype(np.float64)).astype(np.float32)
    off = off + b_off[None, :]
    off = off.reshape(LQ, H, L, P, 2)
    iss_f = iss.astype(np.float32)
    wh = iss_f[:, ::-1]                                       # (L, 2) = (W, H)
    loc = rp[:, None, :, None, :] + off / wh[None, None, :, None, :]
    loc = np.clip(loc, np.float32(0.0), np.float32(0.999))
    idx = (loc * iss_f[None, None, :, None, :]).astype(np.int32)
    h_l = iss[:, 0]
    flat = (idx[..., 0] + idx[..., 1] * h_l[None, None, :, None]
            + lvst[None, None, :, None])                      # (Lq, H, L, P)
    return flat


def _host_prepare(inputs):
    """Build per-core input maps from the full problem inputs."""
    q = np.asarray(inputs["query"], np.float32)[0]            # (1024, 256)
    flat_in = np.ascontiguousarray(np.asarray(inputs["input_flatten"], np.float32)[0])
    addk = np.asarray(inputs["add_keys"], np.float32)[0]
    W_attn = np.asarray(inputs["W_attn"], np.float32)
    W_val = np.asarray(inputs["W_val"], np.float32)
    b_val = np.asarray(inputs["b_val"], np.float32)
    W_mix = np.asarray(inputs["W_mix"], np.float32)

    flat = _host_indices(inputs)                              # (Lq, H, L, P)

    ones128 = np.ones((128, 1), np.float32)
    flat16 = flat_in.astype(np.float16)
    f3pad = np.zeros((256, C), np.float16)
    f3pad[:LEN_IN - 13125] = flat16[13125:]
    common = {
        "qT16": np.ascontiguousarray(q.T).astype(np.float16),
        "flatten16": flat16,
        "addkT": np.ascontiguousarray(addk.T),
        "ident": np.eye(128, dtype=np.float32),
        "f3pad": f3pad,
    }
    # diag extraction mask: rows r=(ql%2)*64+t, cols p*64+t' -> 1 iff t'==r%64
    dm = np.zeros((128, 512), np.float32)
    for rr in range(128):
        dm[rr, rr % 64::64] = 1.0
    common["dmask"] = dm
    oh = np.zeros((128, 8), np.float32)
    for rr in range(128):
        oh[rr, rr // 64] = 1.0          # ich 0: i//64 = p
        oh[rr, 4 + 2 + rr // 64] = 1.0  # ich 1: p = 2 + i'//64
    common["oh48"] = oh

    in_maps = []
    for h in range(H):
        fl = flat[:, h].reshape(LQ, NJ).astype(np.int16)      # (1024, 16) j=4*lvl+p
        idx16 = np.zeros((16, 768), np.int16)
        for t in range(NT):
            blk = fl[128 * t:128 * t + 128, 0:8]              # levels 0-1
            idx16[:, 64 * t:64 * t + 64] = blk.T.ravel().reshape(64, 16).T
            blk2 = fl[128 * t:128 * t + 128, 8:12]            # level 2
            idx16[:, 512 + 32 * t:512 + 32 * t + 32] = \
                blk2.T.ravel().reshape(32, 16).T
        idx16 = np.tile(idx16, (8, 1))  # Q7 tx/rx cores read their own 16-group
        fl3 = fl[:, 12:16].astype(np.int32) - 13125           # level-3 local idx
        oh3 = np.zeros((128, 32, 2, 128), np.float16)
        qa = np.arange(128)
        for t in range(NT):
            for jj in range(4):
                v = fl3[128 * t:128 * t + 128, jj]
                oh3[v % 128, 4 * t + jj, v // 128, qa] = 1.0
        oh3 = oh3.reshape(128, 8192)
        order = [h, 8] + [k for k in range(9) if k not in (h, 8)]
        m = dict(common)
        m["idx16"] = idx16
        m["oh3"] = oh3
        m["wattnT"] = np.ascontiguousarray(
            np.transpose(W_attn[4 * h:4 * h + 5], (0, 2, 1)))
        m["wvT16"] = np.ascontiguousarray(W_val[2 * h].T).astype(np.float16)
        m["wv2T"] = np.ascontiguousarray(W_val[2 * h + 1].T)
        m["bvd"] = (b_val[2 * h] - b_val[2 * h + 1]).reshape(1, C).astype(np.float32)
        m["bv2"] = b_val[2 * h + 1].reshape(C, 1).astype(np.float32)
        m["wmix_r"] = np.ascontiguousarray(W_mix[:, order])
        m["flag"] = ones128 * (1.0 if h == 0 else 0.0)
        in_maps.append(m)
    return in_maps


_CACHE = {}


def _get_nc():
    if "nc" not in _CACHE:
        nc = bacc.Bacc("TRN2", target_bir_lowering=False, debug=False)
        with tile.TileContext(nc) as tc:
            with ExitStack() as ctx:
                build_kernel(nc, tc, ctx)
        nc.compile()
        _CACHE["nc"] = nc
    return _CACHE["nc"]


def kernel(**inputs):
    nc = _get_nc()
    in_maps = _host_prepare(inputs)
    res = run_bass_kernel_spmd(nc, in_maps, core_ids=list(range(8)))
    total = np.zeros((C, LQ), np.float32)
    for h in range(H):
        total = total + res.results[h]["outT"]
    return np.ascontiguousarray(total.T)[None].astype(np.float32)
